# revision 18
# baseline (speedup 1.0000x reference)
"""Multi-head attention (B=2, S=2048, E=1024, H=16, D=64) on 8 TRN2 cores.

Sharding: core c handles batch b = c//4 and head-group g = c%4 (4 heads,
256 embed cols). No cross-core communication; host slices inputs (pre-
transposed, pre-shuffled for contiguous DMA lines, pre-cast to bf16) and
gathers/normalizes outputs.

Per-core device program (f16 matmuls, fp32 PSUM accumulation), paced so
the ScalarE exp stream (128 x [128,1024] ACTIVATEs ~ 1us each) never
starves -- it is the hard floor of the kernel:
  - inputs stream in S-block-major order (K first, then Q sb0) so the
    first scores matmul fires after ~3.5MB of DMA instead of the full
    14MB; V and the remaining Q/K chunks arrive during attention.
  - prologue = K-proj ch0/sb0 + Q-proj ch0/sb0 only; all other
    projection chunks are emitted as budget-paced fillers inside the
    attention steps' PE slack, with hard ensure() ordering at each
    consumer boundary.
  - attention processes head PAIRS: two scores matmuls run concurrently
    on disjoint PE row groups (K=64 each) into one [128,1024] PSUM tile;
    one ScalarE exp (scale=1/8 fused) covers both heads per step.
  - vh carries a ones column (m=64), so the out-stage accumulates the
    softmax denominator in PSUM row 64; host divides.
  - PSUM budget (8 banks): scores 2x[128,1024] (4) + out pair 2x[128,512]
    (2) + projection fillers 2x[128,512] (2).
"""

import sys

sys.path.insert(0, "/opt/trn_rl_repo")

import os

import numpy as np

if os.environ.get("JAX_PLATFORMS") == "cpu":
    # the bass program must run on the neuron cores; the axon/neuron PJRT
    # platform registers only when JAX_PLATFORMS is unset/empty
    del os.environ["JAX_PLATFORMS"]

import concourse.bass as bass  # noqa: F401
import concourse.mybir as mybir
from concourse import bacc
from concourse.tile import TileContext

B, S, E = 2, 2048, 1024
H, D = 16, 64
HPC = 4  # heads per core
COLS = HPC * D  # 256
P = 128
F32 = mybir.dt.float32
F16 = mybir.dt.float16
ET = E // P  # 8 e-tiles
JT = S // P  # 16 j-tiles
NB = 512
NIQ = S // NB  # 4 i-quarters
SB = 4  # s-blocks of 512 for input streaming

# ---- emission pacing knobs (all in measured wall ns) ----
STEP_NS = 1030.0  # ACT exp cadence per step
CAP_NS = 1100.0  # PE budget per step; < ~1147 keeps the ACT pipe overlapped
SCORES_NS = 250.0
OUT_NS = 490.0
LAG0 = 20  # out-matmul lag behind exp, early
LAG_MIN = 5
DRAIN_IT = 84  # start shrinking lag here
EXP_BUFS = LAG0 + 4
DMA_T0 = 9500.0  # observed dead time before any DMA data flows
DMA_BW_SOLO = 300.0  # B/ns while the sync ring runs alone
DMA_BW_SHARED = 210.0  # B/ns per ring once the V ring opens
DMA_GEN = 6.5  # ns per descriptor row (HWDGE gen)
MARGIN = 1000.0  # ns safety before trusting a DMA estimate

_CACHED = {}


def build():
    nc = bacc.Bacc("TRN2", target_bir_lowering=False, debug=False)
    # host pre-shuffled layouts (see _prep_in_maps):
    #   xr[p, sb*8*512 + et*512 + so] = x[b][sb*512+so, et*128+p]
    #   wr[p, et*256 + c]             = w[et*128+p, c-slice of this core]
    qr = nc.dram_tensor("qr", [P, SB * ET * NB], F16, kind="ExternalInput")
    kr = nc.dram_tensor("kr", [P, SB * ET * NB], F16, kind="ExternalInput")
    vr = nc.dram_tensor("vr", [P, SB * ET * NB], F16, kind="ExternalInput")
    wq = nc.dram_tensor("wq", [P, ET * COLS], F16, kind="ExternalInput")
    wk = nc.dram_tensor("wk", [P, ET * COLS], F16, kind="ExternalInput")
    wv = nc.dram_tensor("wv", [P, ET * COLS], F16, kind="ExternalInput")
    bq = nc.dram_tensor("bq", [P, 2], F32, kind="ExternalInput")
    # out_raw[:, (h*NIQ+iq)*NB : ...]: rows 0-63 numerator (d), row 64 denom
    out_raw = nc.dram_tensor("out_raw", [65, HPC * S], F32,
                             kind="ExternalOutput")  # [65, 8192]

    with TileContext(nc) as tc:
        with (
            tc.tile_pool(name="wp", bufs=1) as wp,
            tc.tile_pool(name="xp", bufs=1) as xp,
            tc.tile_pool(name="hp", bufs=1) as hp,
            tc.tile_pool(name="pe", bufs=EXP_BUFS) as pe,
            tc.tile_pool(name="ob", bufs=4) as ob,
            tc.tile_pool(name="psA", bufs=2, space="PSUM") as psA,
            tc.tile_pool(name="psO", bufs=2, space="PSUM") as psO,
            tc.tile_pool(name="psF", bufs=2, space="PSUM") as psF,
        ):
            wk_b = wp.tile([P, ET * COLS], F16)
            wq_b = wp.tile([P, ET * COLS], F16)
            wv_b = wp.tile([P, ET * COLS], F16)
            bq_t = wp.tile([P, 2], F32)
            warm = wp.tile([P, 2], F16)
            kx = xp.tile([P, SB * ET * NB], F16, name="kx")
            qx = xp.tile([P, SB * ET * NB], F16, name="qx")
            vx = xp.tile([P, SB * ET * NB], F16, name="vx")
            qhT = hp.tile([P, 2, S], F16)  # [2 heads x 64 c, chunk, s]
            khT = hp.tile([P, 2, S], F16)
            vh_aug = hp.tile([P, JT, HPC * 65], F16)

            # ---- DMA issue + readiness model ----
            # Critical path (weights, K, Q) on the sync ring in deadline
            # order.  V goes on the gpsimd ring, but GATED behind a junk
            # gpsimd op that depends on the first K-projection evac -- so V
            # cannot steal packet-interleaved HBM bandwidth from the
            # prologue, yet still lands long before the out-stage needs it.
            ready = {}
            clk = {"gen": DMA_T0, "xfer": DMA_T0}
            V_GATE = 17000.0  # when the V ring opens (junk op fires)

            def dma(key, dst, src, nbytes, lines, bw):
                nc.sync.dma_start(dst, src)
                clk["gen"] += lines * DMA_GEN + 100.0
                clk["xfer"] = max(clk["gen"], clk["xfer"]) + nbytes / bw
                ready[key] = clk["xfer"]

            def sbs(i):  # s-block slice of a shuffled input row
                return slice(i * ET * NB, (i + 1) * ET * NB)

            XB = P * ET * NB * 2  # bytes per s-block piece (1MB)
            WB = P * ET * COLS * 2
            dma("bq", bq_t, bq[:, :], 1024, P, DMA_BW_SOLO)
            dma("wk", wk_b, wk[:, :], WB, P, DMA_BW_SOLO)
            dma("wq", wq_b, wq[:, :], WB, P, DMA_BW_SOLO)
            dma("k0", kx[:, sbs(0)], kr[:, sbs(0)], XB, 2 * P, DMA_BW_SOLO)
            dma("q0", qx[:, sbs(0)], qr[:, sbs(0)], XB, 2 * P, DMA_BW_SOLO)
            dma("k1", kx[:, sbs(1)], kr[:, sbs(1)], XB, 2 * P, DMA_BW_SOLO)
            dma("k2", kx[:, sbs(2)], kr[:, sbs(2)], XB, 2 * P, DMA_BW_SHARED)
            dma("k3", kx[:, sbs(3)], kr[:, sbs(3)], XB, 2 * P, DMA_BW_SHARED)
            dma("q1", qx[:, sbs(1)], qr[:, sbs(1)], XB, 2 * P, DMA_BW_SHARED)
            dma("q2", qx[:, sbs(2)], qr[:, sbs(2)], XB, 2 * P, DMA_BW_SHARED)
            dma("q3", qx[:, sbs(3)], qr[:, sbs(3)], XB, 2 * P, DMA_BW_SHARED)

            # ---- ACT warmup: preload the exp table during the DMA wait ----
            nc.scalar.activation(
                warm, bq_t, mybir.ActivationFunctionType.Exp, scale=0.0
            )
            nc.vector.memset(vh_aug, 1.0)

            # ---- PE warmup: junk matmuls bridging the DMA wait so HAM stays
            # unthrottled when the first projection chunk lands ----
            NWARM = 24
            wps = psF.tile([P, NB], F32, tag="f", name="warmps")
            for i in range(NWARM):
                nc.tensor.matmul(
                    wps[:, 0:P],
                    wk_b[:, (i % 2) * P : (i % 2 + 1) * P],
                    wk_b[:, 0:P],
                    start=(i == 0),
                    stop=(i == NWARM - 1),
                )

            # ---- projection chunk generators (each yields ~PE cycles) ----
            def x_sl(x, sb, et, c0, cn):  # rhs slice of shuffled input
                base = (sb * ET + et) * NB
                return x[:, base + c0 : base + c0 + cn]

            def qk_chunk(w_b, x, dst, bias, ch, sb):
                ps = psF.tile([P, NB], F32, tag="f", name=f"ps{ch}{sb}")
                for et0 in range(0, ET, 2):
                    for et in (et0, et0 + 1):
                        nc.tensor.matmul(
                            ps,
                            w_b[:, et * COLS + ch * P : et * COLS + (ch + 1) * P],
                            x_sl(x, sb, et, 0, NB),
                            start=(et == 0),
                            stop=(et == ET - 1),
                        )
                    yield 490.0  # measured ns for 2 N=512 matmuls
                if bias is not None:
                    nc.vector.tensor_scalar_add(
                        dst[:, ch, sb * NB : (sb + 1) * NB], ps, bias[:, ch : ch + 1]
                    )
                else:
                    nc.vector.tensor_copy(dst[:, ch, sb * NB : (sb + 1) * NB], ps)
                yield 0

            def v_chunk(sc):
                ps = psF.tile([P, NB], F32, tag="f", name=f"psv{sc}")
                for et0 in range(0, ET, 4):
                    for et in range(et0, et0 + 4):
                        nc.tensor.matmul(
                            ps[:, :COLS],
                            x_sl(vx, sc // 4, et, (sc % 4) * P, P),
                            wv_b[:, et * COLS : (et + 1) * COLS],
                            start=(et == 0),
                            stop=(et == ET - 1),
                        )
                    yield 460.0  # measured ns for 4 N=256 matmuls
                nc.vector.tensor_copy(
                    vh_aug[:, sc].rearrange("p (h x) -> p h x", x=65)[:, :, :D],
                    ps[:, :COLS].rearrange("p (h x) -> p h x", x=D),
                )
                yield 0

            # ---- filler queue: (key, ready-key, generator) ----
            def fillq():
                for sb in (1, 2, 3):
                    yield f"kh0_{sb}", "k" + str(sb), qk_chunk(wk_b, kx, khT, None, 0, sb)
                yield "qh0_1", "q1", qk_chunk(wq_b, qx, qhT, bq_t, 0, 1)
                for sc in range(4):
                    yield f"vh_{sc}", f"v{sc // 4}", v_chunk(sc)
                yield "qh0_2", "q2", qk_chunk(wq_b, qx, qhT, bq_t, 0, 2)
                for sc in range(4, 8):
                    yield f"vh_{sc}", f"v{sc // 4}", v_chunk(sc)
                yield "qh0_3", "q3", qk_chunk(wq_b, qx, qhT, bq_t, 0, 3)
                for sc in range(8, 16):
                    yield f"vh_{sc}", f"v{sc // 4}", v_chunk(sc)
                for sb in range(4):
                    yield f"kh1_{sb}", "k" + str(sb), qk_chunk(wk_b, kx, khT, None, 1, sb)
                for sb in range(4):
                    yield f"qh1_{sb}", "q" + str(sb), qk_chunk(wq_b, qx, qhT, bq_t, 1, sb)

            fillers = list(fillq())
            done = set()

            def pull_one():
                """Emit one quantum from the head filler; True if emitted."""
                if not fillers:
                    return 0.0
                key, rkey, gen = fillers[0]
                try:
                    return float(next(gen)) + 1.0
                except StopIteration:
                    done.add(key)
                    fillers.pop(0)
                    return 1.0

            def ensure(key):
                while fillers and key not in done:
                    pull_one()

            def run_gen(gen):
                for _ in gen:
                    pass

            # ---- prologue: K ch0 sb0 + Q ch0 sb0 only ----
            run_gen(qk_chunk(wk_b, kx, khT, None, 0, 0))
            # open the V ring: a junk gpsimd op that waits on the first K
            # evac keeps the V DMA packets off the prologue's critical path
            vgate = wp.tile([P, 8], F16)
            nc.gpsimd.tensor_copy(vgate, khT[:, 0, 0:8])
            gclk = {"gen": V_GATE, "xfer": V_GATE}

            def gdma(key, dst, src, nbytes, lines):
                nc.gpsimd.dma_start(dst, src)
                gclk["gen"] += lines * 12.0 + 500.0
                gclk["xfer"] = max(gclk["gen"], gclk["xfer"]) + nbytes / DMA_BW_SHARED
                ready[key] = gclk["xfer"]

            gdma("wv", wv_b, wv[:, :], WB, P)
            for i in range(SB):
                gdma(f"v{i}", vx[:, sbs(i)], vr[:, sbs(i)], XB, 2 * P)

            run_gen(qk_chunk(wq_b, qx, qhT, bq_t, 0, 0))
            done.add("kh0_0")
            done.add("qh0_0")
            est0 = max(ready["q0"] + 2500.0, ready["k0"] + 4500.0)

            # ---- attention steps; scores+exp lead, outs lag, fillers fill ----
            from collections import deque

            steps = [(pr, iq, jt) for pr in range(2) for iq in range(NIQ)
                     for jt in range(JT)]
            pending = deque()
            ops = {}

            def emit_out(pr, iq, jt, expT):
                ensure(f"vh_{jt}")
                if jt == 0:
                    ops[(pr, iq)] = (
                        psO.tile([P, NB], F32, tag="o", name="op0"),
                        psO.tile([P, NB], F32, tag="o", name="op1"),
                    )
                op0, op1 = ops[(pr, iq)]
                for hh, op in ((0, op0), (1, op1)):
                    h = 2 * pr + hh
                    nc.tensor.matmul(
                        op[:65, :],
                        vh_aug[:, jt, h * 65 : (h + 1) * 65],
                        expT[:, hh * NB : (hh + 1) * NB],
                        start=(jt == 0),
                        stop=(jt == JT - 1),
                    )
                if jt == JT - 1:  # evacuate + store this iq's outputs
                    for hh, op in ((0, op0), (1, op1)):
                        r = (2 * pr + hh) * NIQ + iq
                        osb = ob.tile([P, NB], F32, tag="ob", name="osb")
                        nc.vector.tensor_copy(osb[:65, :], op[:65, :])
                        nc.sync.dma_start(
                            out_raw[:, r * NB : (r + 1) * NB], osb[:65, :]
                        )
                    del ops[(pr, iq)]

            def target_lag(it):
                if it < DRAIN_IT:
                    return LAG0
                return max(LAG_MIN, LAG0 - (it - DRAIN_IT + 1) // 2)

            budget = 0.0
            for it, (pr, iq, jt) in enumerate(steps):
                # hard deps for this step's scores
                if jt % 4 == 0:
                    ensure(f"kh{pr}_{jt // 4}")
                if jt == 0:
                    ensure(f"qh{pr}_{iq}")
                sps = psA.tile([P, 2 * NB], F32, tag="s", name="sps")
                for hh in range(2):  # row-group packed, concurrent
                    r0 = hh * D
                    nc.tensor.matmul(
                        sps[:, hh * NB : (hh + 1) * NB],
                        khT[r0 : r0 + D, pr, jt * P : (jt + 1) * P],
                        qhT[r0 : r0 + D, pr, iq * NB : (iq + 1) * NB],
                        start=True,
                        stop=True,
                    )
                expT = pe.tile([P, 2 * NB], F16, tag="e", name="expT")
                nc.scalar.activation(
                    expT, sps, mybir.ActivationFunctionType.Exp, scale=0.125
                )
                pending.append((pr, iq, jt, expT))
                emitted = SCORES_NS
                while len(pending) > target_lag(it):
                    emit_out(*pending.popleft())
                    emitted += OUT_NS
                budget = min(budget + CAP_NS - emitted, 2.0 * CAP_NS)
                est_now = est0 + it * STEP_NS
                while budget > 0.0 and fillers:
                    rkey = fillers[0][1]
                    if ready.get(rkey, 0.0) + MARGIN > est_now:
                        break
                    q = pull_one()
                    budget -= q
                # once projections are done, spend spare budget draining the
                # out-lag so the post-loop tail stays short
                while not fillers and budget >= OUT_NS and len(pending) > LAG_MIN:
                    emit_out(*pending.popleft())
                    budget -= OUT_NS
            while fillers:
                pull_one()
            while pending:
                emit_out(*pending.popleft())
    nc.finalize()
    return nc


def _prep_in_maps(q, k, v, wq, bq, wk, bk, wv, bv):
    bf = np.float16
    q, k, v = (np.asarray(x, np.float32) for x in (q, k, v))
    wqb, wkb, wvb = (np.asarray(x, bf) for x in (wq, wk, wv))
    bq = np.asarray(bq, np.float32)

    def shuf_x(xb):  # [S, E] f32 -> [128, SB*ET*NB] f16 s-block-major
        xT = np.ascontiguousarray(xb.T.astype(bf))  # [E, S]
        return np.ascontiguousarray(
            xT.reshape(ET, P, SB, NB).transpose(1, 2, 0, 3).reshape(P, SB * ET * NB)
        )

    def shuf_w(wb):  # [E, COLS] f16 -> [128, ET*COLS]
        return np.ascontiguousarray(
            wb.reshape(ET, P, COLS).transpose(1, 0, 2).reshape(P, ET * COLS)
        )

    qs = [shuf_x(q[b]) for b in range(B)]
    ks = [shuf_x(k[b]) for b in range(B)]
    vs = [shuf_x(v[b]) for b in range(B)]
    in_maps = []
    for c in range(8):
        b, g = divmod(c, 4)
        cs = slice(g * COLS, (g + 1) * COLS)
        in_maps.append(
            {
                "qr": qs[b],
                "kr": ks[b],
                "vr": vs[b],
                "wq": shuf_w(wqb[:, cs]),
                "wk": shuf_w(wkb[:, cs]),
                "wv": shuf_w(wvb[:, cs]),
                "bq": np.ascontiguousarray(bq[cs].reshape(2, P).T),
            }
        )
    return in_maps


def _make_runner(nc, n_cores=8):
    """Persistent jitted shard_map runner over the prebuilt Bass module."""
    import jax
    from jax.experimental.shard_map import shard_map
    from jax.sharding import Mesh, NamedSharding, PartitionSpec
    from concourse import bass2jax

    bass2jax.install_neuronx_cc_hook()

    in_names, out_names, out_avals, zero_outs = [], [], [], []
    for alloc in nc.m.functions[0].allocations:
        if not isinstance(alloc, mybir.MemoryLocationSet):
            continue
        name = alloc.memorylocations[0].name
        if alloc.kind == "ExternalInput":
            in_names.append(name)
        elif alloc.kind == "ExternalOutput":
            shape = tuple(alloc.tensor_shape)
            dtype = mybir.dt.np(alloc.dtype)
            out_avals.append(jax.core.ShapedArray(shape, dtype))
            zero_outs.append(np.zeros((n_cores * shape[0], *shape[1:]), dtype))
            out_names.append(name)
    pid_name = nc.partition_id_tensor.name if nc.partition_id_tensor else None
    if pid_name is not None:
        in_names = [n for n in in_names if n != pid_name]
    n_params = len(in_names)
    all_names = in_names + out_names + ([pid_name] if pid_name else [])

    def _body(*args):
        operands = list(args)
        if pid_name is not None:
            operands.append(bass2jax.partition_id_tensor())
        outs = bass2jax._bass_exec_p.bind(
            *operands,
            out_avals=tuple(out_avals),
            in_names=tuple(all_names),
            out_names=tuple(out_names),
            lowering_input_output_aliases=(),
            sim_require_finite=True,
            sim_require_nnan=True,
            nc=nc,
        )
        return tuple(outs)

    devices = jax.devices()[:n_cores]
    mesh = Mesh(np.asarray(devices), ("core",))
    nio = n_params + len(out_names)
    sharded = jax.jit(
        shard_map(
            _body,
            mesh=mesh,
            in_specs=(PartitionSpec("core"),) * nio,
            out_specs=(PartitionSpec("core"),) * len(out_names),
            check_rep=False,
        ),
        keep_unused=True,
    )
    row_sharding = NamedSharding(mesh, PartitionSpec("core"))
    zeros_dev = [jax.device_put(z, row_sharding) for z in zero_outs]

    def run(in_maps):
        concat_in = [
            np.concatenate([np.asarray(m[name]) for m in in_maps], axis=0)
            for name in in_names
        ]
        out_arrs = sharded(*concat_in, *zeros_dev)
        return [
            {
                name: np.asarray(out_arrs[i]).reshape(n_cores, *out_avals[i].shape)[c]
                for i, name in enumerate(out_names)
            }
            for c in range(n_cores)
        ]

    run.sharded = sharded
    run.in_names = in_names
    run.zeros_dev = zeros_dev
    run.row_sharding = row_sharding
    return run


def get_runner():
    if "run" not in _CACHED:
        _CACHED["nc"] = build()
        _CACHED["run"] = _make_runner(_CACHED["nc"])
    return _CACHED["run"]


def kernel(q, k, v, wq, bq, wk, bk, wv, bv):
    run = get_runner()
    in_maps = _prep_in_maps(q, k, v, wq, bq, wk, bk, wv, bv)
    results = run(in_maps)

    bv = np.asarray(bv, np.float32)
    out = np.empty((B, S, E), np.float32)
    for c in range(8):
        b, g = divmod(c, 4)
        raw = results[c]["out_raw"]  # [65, 8192]
        num = raw[:64].reshape(64, HPC, S)  # [d, h, i] (NIQ*NB = S)
        den = raw[64].reshape(HPC, S)
        for h in range(HPC):
            col0 = g * COLS + h * D
            o = num[:, h, :] / den[h][None, :]
            out[b, :, col0 : col0 + D] = o.T + bv[col0 : col0 + D][None, :]
    return out
